# revision 25
# baseline (speedup 1.0000x reference)
"""NF5 (32-level NormalFloat) block-quantized linear layer on 8 TRN2 NeuronCores.

y[b,s,o] = sum_k Q(x)[b,s,k] * Q(w)[o,k] + bias[o]

Q = block-32 NF5 quantize-dequantize with power-of-2 scales.

Sharding: data-parallel over batch (core i handles batch i). The small
1024x1024 weight is quantized once on the host (exact reference semantics),
cast to bf16, pre-transposed to [k, o], and replicated to all 8 cores, per
the sharding hint ("replicate the small 1024x1024 quantized weight").

Per-core pipeline (engine-balanced, groups of 4 row-tiles = [128, 4096]):
  - Pool:  blockwise absmax (tensor_reduce), power-of-2 scale bit tricks,
           n = x * inv_scale, q = t * scale (bf16 out)
  - ACT:   u = erf(ERFS*n), u2 = prelu(u, alpha)
  - DVE:   3 custom ops/elem: M7 (odd-deg-7 index poly, T7-shaped),
           KV2B (even m^2 correction + magic round + sign-dependent
           dequant-input scale, one fused op), T7 (odd-deg-7 dequant poly)
  - SP/ACT HWDGE: qx [s,k] -> qxT [k,s] via 128x128 DMA-engine transposes
           (no PE transposes, no PSUM eviction for qx)
  - PE:    pure bf16 matmuls, yT layout: out[o-tile, s-chunk] so bias is a
           per-partition scalar
  - evict: ACT Identity(+bias AP) or DVE tensor_scalar(+bias AP) from PSUM,
           then DMA to a [1024, 4096] yT output (host transposes back)
"""

import numpy as np

import concourse.bacc as bacc
import concourse.bass as bass
import concourse.mybir as mybir
from concourse.tile import TileContext
from concourse.bass_utils import run_bass_kernel_spmd

# ---------------------------------------------------------------------------
# quantization constants
BLOCK = 32
NF5_OFFSET = 0.9677083
ERFS = 1.306825934165241      # ndtri(0.9677083)/sqrt(2)
ALPHA = 16.0 / 15.0           # negative-side prelu slope (SN/SP)
MAGIC = 12582912.0            # 1.5 * 2**23 (round-to-nearest-even)
DPN = 0.029231768749999998
DPP = 0.03118055333333333

# x index chain: m = u2*(A1 + A3 u2^2 + A5 u2^4 + A7 u2^6); m2 = m + CE*m^2;
# k = (m2 + MAGIC) - MAGIC  (density-weighted crossing fit, see fit_m7.py)
XA1 = 16.02426431807089
XA3 = -0.020928588694994074
XA5 = 0.05613237111203201
XA7 = -0.11446199992368593
XCE = -3.442109055933305e-05

# dequant: t = v*(TX0 + TX1 w + TX2 w^2 + TX3 w^3), w = v*v
TX = [1.3394853079428255, 2.6522251754182475, -16.231571777775724,
      94.1865421528677]

B, S, DIN, DOUT = 8, 4096, 1024, 1024
P = 128
GT = 2                        # row-tiles per group
GE = GT * DIN                 # free elems per group tile (4096)
NG = S // (GT * P)            # 8 groups per core
NBLK = GE // BLOCK            # 128 blocks per group row
KT = DIN // P                 # 8 k-tiles
OT = DOUT // P                # 8 o-tiles
SC = GT * P                   # s-chunk per group (512)
F32 = mybir.dt.float32
BF16 = mybir.dt.bfloat16

# ---------------------------------------------------------------------------
# custom DVE ops
_OPS_REGISTERED = {}


def _register_ops():
    if _OPS_REGISTERED:
        return _OPS_REGISTERED
    import concourse.dve_ops as dops
    from concourse.dve_spec import (
        Spec, Src0, Src1, C0, C1, C2, C3, Zero, select, sq,
        lower, _has_src1, _spill_c3_to_src1,
    )
    from concourse.dve_uop import DveOpSpec

    def mk(name, spec):
        if name in dops._SUB_OPCODE_FOR_NAME:
            op = next(o for o in dops.OPS if o.name == name)
            _OPS_REGISTERED[name] = op
            return op
        row = dops._CUSTOM_DVE_ROW_BASE + len(dops.OPS)
        assert row < 0x20, "custom DVE row overflow"
        shas = {}
        for ver in ("v3", "v4"):
            uops = lower(spec, ver=ver)
            shas[ver] = DveOpSpec(
                name=name, opcode=row, uops=uops, rd1_en=_has_src1(spec)
            ).sha(ver)
        op = dops.DveOp(name, spec, subdim=False, uops_sha=shas)
        dops.OPS.append(op)
        dops._SUB_OPCODE_FOR_NAME[name] = row
        dops.CUSTOM_DVE_SPECS[name] = spec
        _OPS_REGISTERED[name] = op
        return op

    # t = (((c3*w + s0)*w + s1)*w + imm2) * v, w = v*v  (odd deg-7, C3 via in1)
    # used both for the index poly (M7) and the dequant poly (T7)
    _w7 = sq(Src0)
    mk("NF5_T7", Spec(
        body=_spill_c3_to_src1(
            (((C3 * _w7 + C0) * _w7 + C1) * _w7 + C2) * Src0),
        reference=lambda in0, in1, s0, s1, imm2:
            ((((in1[:, :1] * (in0 * in0) + s0) * (in0 * in0) + s1)
              * (in0 * in0) + imm2) * in0).astype(np.float32),
    ))

    # m2 = m + c3*m^2 ; k = (m2 + s0) - s0 (magic round) ;
    # v = k * (k<0 ? s1 : imm2)
    def _kv2b_ref(in0, in1, s0, s1, imm2):
        m2 = (in0 + (in1[:, :1] * in0).astype(np.float32) * in0
              ).astype(np.float32)
        k = ((m2 + np.float32(s0)).astype(np.float32)
             - np.float32(s0)).astype(np.float32)
        return (k * np.where(k < 0, np.float32(s1), np.float32(imm2))
                ).astype(np.float32)
    _m2 = Src0 + C3 * sq(Src0)
    _kk = (_m2 + C0) - C0
    mk("NF5_KV2B", Spec(
        body=_spill_c3_to_src1(_kk * select(_kk < Zero, C1, C2)),
        reference=_kv2b_ref,
    ))

    # cp = round(((lo>0) + hi + s0) * s1)  via magic add/sub (imm2=MAGIC)
    def _cp_ref(in0, in1, s0, s1, imm2):
        s = ((in0 > 0).astype(np.float32) + in1 + np.float32(s0)
             ).astype(np.float32)
        s = (s * np.float32(s1)).astype(np.float32)
        s = (s + np.float32(imm2)).astype(np.float32)
        return (s - np.float32(imm2)).astype(np.float32)
    mk("NF5_CP", Spec(
        body=((Src0 > Zero) + Src1 + C0) * C1 + C2 - C2,
        reference=_cp_ref,
    ))
    return _OPS_REGISTERED


# ---------------------------------------------------------------------------
def _build_nc():
    _register_ops()
    ops = _OPS_REGISTERED
    nc = bacc.Bacc("TRN2", target_bir_lowering=False, num_devices=B)
    x = nc.dram_tensor("x", (S, DIN), F32, kind="ExternalInput")
    qwt = nc.dram_tensor("qwt", (DIN, DOUT), BF16, kind="ExternalInput")
    bvec = nc.dram_tensor("b", (DOUT,), F32, kind="ExternalInput")
    yt = nc.dram_tensor("yt", (DOUT, S), F32, kind="ExternalOutput")

    U16 = mybir.dt.uint16

    with TileContext(nc) as tc:
        from contextlib import ExitStack
        with ExitStack() as ctx:
            const_pool = ctx.enter_context(tc.tile_pool(name="const", bufs=1))
            xin_pool = ctx.enter_context(tc.tile_pool(name="xin", bufs=4))
            ew_pool = ctx.enter_context(tc.tile_pool(name="ew", bufs=2))
            sc_pool = ctx.enter_context(tc.tile_pool(name="sc", bufs=6))
            qx_pool = ctx.enter_context(tc.tile_pool(name="qx", bufs=2))
            qxt_pool = ctx.enter_context(tc.tile_pool(name="qxt", bufs=2))
            yout_pool = ctx.enter_context(tc.tile_pool(name="yout", bufs=10))
            psum_mm = ctx.enter_context(
                tc.tile_pool(name="psmm", bufs=8, space="PSUM"))

            def const1(val, tag):
                t = const_pool.tile([P, 1], F32, tag=tag)
                nc.vector.memset(t[:], float(val))
                return t[:]

            c_xa7 = const1(XA7, "c_xa7")
            c_xce = const1(XCE, "c_xce")
            c_tx3 = const1(TX[3], "c_tx3")

            # persistent transposed quantized weight [p=k_local, kt, o]
            qwT = const_pool.tile([P, KT * DOUT], BF16)
            nc.gpsimd.dma_start(
                qwT[:].rearrange("p (kt o) -> p kt o", kt=KT),
                qwt[:, :].rearrange("(kt p) o -> p kt o", p=P))
            # bias as per-partition columns: bias_sb[p, ot] = bias[ot*128+p]
            bias_sb = const_pool.tile([P, OT], F32)
            nc.gpsimd.dma_start(bias_sb[:],
                              bvec[:].rearrange("(a p) -> p a", p=P))

            HB = GE // 2              # half-group free elems (2048)
            HBLK = NBLK // 2          # blocks per half (64)

            xgs, scls, invs = {}, {}, {}

            def stage_load(g):
                xg = xin_pool.tile([P, GE], F32, tag="xg")
                xgs[g] = xg
                for h in range(2):
                    # fill optimization: the first two groups' h1 halves go
                    # through ACT's HWDGE queue (idle then), halving the
                    # time until the DVE reduces can saturate
                    eng = nc.scalar if (g < 2 and h == 1) else nc.sync
                    eng.dma_start(
                        xg[:, h * HB:(h + 1) * HB].rearrange(
                            "p (t k) -> p t k", t=GT // 2),
                        x[g * SC + h * SC // 2:
                          g * SC + (h + 1) * SC // 2, :].rearrange(
                            "(t p) k -> p t k", p=P))
                scl = [sc_pool.tile([P, HBLK], F32, tag=f"scl{h}",
                                    name=f"scl{h}") for h in range(2)]
                inv = [sc_pool.tile([P, HBLK], F32, tag=f"inv{h}",
                                    name=f"inv{h}") for h in range(2)]
                scls[g], invs[g] = scl, inv
                for h in range(2):
                    nc.gpsimd.memset(scl[h][:], 0.0)
                    nc.gpsimd.memset(inv[h][:], 0.0)

            def stage_scale(g):
                # blockwise absmax (DVE-only reduce) and power-of-2 scale
                # written directly into the f32 exponent field: the CP op
                # reads the u16 halves of amax (value-converted) and
                # magic-rounds (1.5*2^30, ulp 128) into scl's hi u16;
                # inv_hi = 32512 - scl_hi on Pool.
                xg = xgs[g]
                for h in range(2):
                    src3 = xg[:, h * HB:(h + 1) * HB].rearrange(
                        "p (B e) -> p B e", e=BLOCK)
                    amax = sc_pool.tile([P, HBLK], F32, tag=f"amax{h}")
                    nc.vector.tensor_reduce(
                        amax[:], src3, axis=mybir.AxisListType.X,
                        op=mybir.AluOpType.max, apply_absolute_value=True)
                    am16 = amax[:].bitcast(U16).rearrange(
                        "p (b two) -> p b two", two=2)
                    s16 = scls[g][h][:].bitcast(U16).rearrange(
                        "p (b two) -> p b two", two=2)
                    nc.vector._custom_dve(ops["NF5_CP"], out=s16[:, :, 1],
                                          in0=am16[:, :, 0],
                                          in1=am16[:, :, 1],
                                          s0=63.5, s1=1.0,
                                          imm2=1610612736.0)
                    i16 = invs[g][h][:].bitcast(U16).rearrange(
                        "p (b two) -> p b two", two=2)
                    nc.gpsimd.tensor_scalar(i16[:, :, 1], s16[:, :, 1],
                                            -1.0, 32512.0,
                                            mybir.AluOpType.mult,
                                            mybir.AluOpType.add)

            bufBs = {}

            def stage_chain(g):
                xg = xgs.pop(g)
                scl, inv = scls[g], invs.pop(g)
                bufA = ew_pool.tile([P, GE], F32, tag="bufA")
                bufB = ew_pool.tile([P, GE], F32, tag="bufB")
                bufBs[g] = bufB
                hsl = [slice(h * HB, (h + 1) * HB) for h in range(2)]
                # Pool: both halves' n first (no HOL behind q of h0)
                for h in range(2):
                    inv_b = inv[h][:].unsqueeze(-1).to_broadcast(
                        (P, HBLK, BLOCK))
                    nc.gpsimd.tensor_mul(
                        bufA[:, hsl[h]].rearrange("p (B e) -> p B e",
                                                  e=BLOCK),
                        xg[:, hsl[h]].rearrange("p (B e) -> p B e",
                                                e=BLOCK),
                        inv_b)
                # ACT: erf + prelu per half
                for h in range(2):
                    nc.scalar.activation(bufB[:, hsl[h]], bufA[:, hsl[h]],
                                         mybir.ActivationFunctionType.Erf,
                                         bias=0.0, scale=float(ERFS))
                    nc.scalar.activation(bufA[:, hsl[h]], bufB[:, hsl[h]],
                                         mybir.ActivationFunctionType.Prelu,
                                         bias=0.0, scale=1.0,
                                         alpha=float(ALPHA))
                # DVE: index poly, magic round + dequant-input, dequant poly
                for h in range(2):
                    nA, nB = bufA[:, hsl[h]], bufB[:, hsl[h]]
                    nc.vector._custom_dve(ops["NF5_T7"], out=nB,
                                          in0=nA, in1=c_xa7,
                                          s0=float(XA5), s1=float(XA3),
                                          imm2=float(XA1))
                    nc.vector._custom_dve(ops["NF5_KV2B"], out=nA,
                                          in0=nB, in1=c_xce,
                                          s0=float(MAGIC), s1=float(DPN),
                                          imm2=float(DPP))
                    nc.vector._custom_dve(ops["NF5_T7"], out=nB,
                                          in0=nA, in1=c_tx3,
                                          s0=float(TX[2]), s1=float(TX[1]),
                                          imm2=float(TX[0]))

            pmms = {}

            def stage_q_mm(g):
                # Pool: q = t * scale -> bf16 ; SP: xbar transposes; PE: mm
                scl = scls.pop(g)
                bufB = bufBs.pop(g)
                qx = qx_pool.tile([P, GE], BF16, tag="qx")
                qxT = qxt_pool.tile([P, KT * SC], BF16, tag="qxT")
                hsl = [slice(h * HB, (h + 1) * HB) for h in range(2)]
                for h in range(2):
                    scl_b = scl[h][:].unsqueeze(-1).to_broadcast(
                        (P, HBLK, BLOCK))
                    nc.gpsimd.tensor_mul(
                        qx[:, hsl[h]].rearrange("p (B e) -> p B e", e=BLOCK),
                        bufB[:, hsl[h]].rearrange("p (B e) -> p B e",
                                                  e=BLOCK),
                        scl_b)
                for t in range(GT):
                    for kt in range(KT):
                        nc.sync.dma_start_transpose(
                            qxT[:, kt * SC + t * P: kt * SC + (t + 1) * P],
                            qx[:, t * DIN + kt * P:
                               t * DIN + (kt + 1) * P])
                pg = [psum_mm.tile([P, SC], F32, tag="pmm", name="pmm")
                      for ot in range(OT)]
                # h-major: all h0 column-halves first — they only need the
                # first two tiles' transposes, so the PE starts earlier and
                # the last group's drain shortens
                for h in range(2):
                    hs2 = slice(h * SC // 2, (h + 1) * SC // 2)
                    for ot in range(OT):
                        for kt in range(KT):
                            nc.tensor.matmul(
                                pg[ot][:, hs2],
                                lhsT=qwT[:, kt * DOUT + ot * P:
                                         kt * DOUT + (ot + 1) * P],
                                rhs=qxT[:, kt * SC + h * SC // 2:
                                        kt * SC + (h + 1) * SC // 2],
                                start=(kt == 0), stop=(kt == KT - 1),
                                skip_group_check=True)
                pmms[g] = pg

            ysbs = {}

            def stage_evict_act(g):
                pg = pmms.pop(g)
                ys = []
                for ot in range(OT):
                    ysb = yout_pool.tile([P, SC], F32, tag="ysb")
                    ys.append(ysb)
                    nc.scalar.activation(
                        ysb[:], pg[ot][:],
                        mybir.ActivationFunctionType.Identity,
                        bias=bias_sb[:, ot:ot + 1], scale=1.0)
                ysbs[g] = ys

            def stage_ydma(g):
                ys = ysbs.pop(g)
                for ot in range(OT):
                    nc.sync.dma_start(
                        yt[ot * P:(ot + 1) * P, g * SC:(g + 1) * SC],
                        ys[ot][:])

            # software pipeline; per-iteration emission order keeps every
            # engine's in-order stream free of head-of-line stalls:
            #   ACT: evicts(s-4) then erf/prelu(s-2)
            #   Pool: memsets(s), inv(s-1), n(s-2), q(s-3), ydma(s-4)
            #   DVE: reduce/CP(s-1), chain customs(s-2)
            #   SP: x-load(s), transposes(s-3);  PE: mm(s-3)
            for s in range(NG + 4):
                if 0 <= s - 4 < NG:
                    stage_evict_act(s - 4)
                if s < NG:
                    stage_load(s)
                if 0 <= s - 1 < NG:
                    stage_scale(s - 1)
                if 0 <= s - 2 < NG:
                    stage_chain(s - 2)
                if 0 <= s - 3 < NG:
                    stage_q_mm(s - 3)
                if 0 <= s - 4 < NG:
                    stage_ydma(s - 4)
    nc.finalize()
    return nc


# ---------------------------------------------------------------------------
# host-side exact NF5 quantization of the weight (reference semantics)
def _nf5_table_np():
    from scipy.special import ndtri
    neg = ndtri(np.linspace(1.0 - NF5_OFFSET, 0.5, 17, dtype=np.float64))[:-1]
    pos = ndtri(np.linspace(0.5, NF5_OFFSET, 16, dtype=np.float64))[1:]
    neg = neg / (-neg[0])
    pos = pos / pos[-1]
    return np.concatenate([neg, np.zeros(1), pos]).astype(np.float32)


def host_quantize_w(weight):
    table = _nf5_table_np()
    boundaries = ((table[:-1] + table[1:]) * np.float32(0.5)
                  ).astype(np.float32)
    shp = weight.shape
    xb = weight.astype(np.float32).reshape(-1, BLOCK)
    amax = np.max(np.abs(xb), axis=1, keepdims=True).astype(np.float32)
    scale = np.exp2(np.ceil(np.log2(np.maximum(amax, np.float32(1e-12))))
                    ).astype(np.float32)
    n = (xb / scale).astype(np.float32)
    idx = np.searchsorted(boundaries, n)
    q = (table[idx] * scale).astype(np.float32)
    return q.reshape(shp)


def kernel(x, weight, bias):
    import ml_dtypes
    nc = _build_nc()
    qw = host_quantize_w(np.asarray(weight, dtype=np.float32))
    qwt_bf16 = np.ascontiguousarray(
        qw.astype(ml_dtypes.bfloat16).T)          # [k, o] bf16
    bias_f = np.ascontiguousarray(bias).astype(np.float32)
    in_maps = [
        {"x": np.ascontiguousarray(x[i]).astype(np.float32),
         "qwt": qwt_bf16,
         "b": bias_f}
        for i in range(B)
    ]
    res = run_bass_kernel_spmd(nc, in_maps, core_ids=list(range(B)))
    out = np.stack([np.ascontiguousarray(r["yt"].T) for r in res.results],
                   axis=0)
    return out.astype(np.float32)


if __name__ == "__main__":
    rng = np.random.default_rng(0)
    x = rng.standard_normal((B, S, DIN), dtype=np.float32)
    w = ((rng.random((DOUT, DIN), dtype=np.float32) * 2 - 1) / 32.0)
    bvec = ((rng.random(DOUT, dtype=np.float32) * 2 - 1) / 32.0)
    y = kernel(x, w, bvec)
    print(y.shape, y.dtype)
